# revision 7
# baseline (speedup 1.0000x reference)
"""Trainium2 Bass kernel for the CropHealthClassifier SNN.

Model: population-encoded Bernoulli spike trains -> 2-layer LIF SNN over
T=100 timesteps -> output firing rates (B, C).

Strategy (pure data parallel over 8 NeuronCores, B=4096 -> 512/core):
- Rescaled-membrane formulation: track m~_t = m_t / beta^t so the PE can
  accumulate layer inputs directly into a PSUM-resident membrane state
  (matmul start=False accumulates; ACT/DVE writes to PSUM don't clear the
  has_written bits, so the accumulate-onto-modified-state pattern works).
  The per-step beta^-t input scale is folded into per-step weight rescales
  (one ACT copy op) and the spike threshold becomes theta_t = beta^-t
  (per-step bias on the Sign activation).
- Layout B: membrane m1~ is (H=512 -> free-dim-tiled, Bc=512) in PSUM,
  spikes enter as (D, Bc) via PE transposes of the noise, the layer-2
  matmul consumes sgn tiles (H, Bc) directly as the moving operand.
- Spike sign trick: ACT Sign gives sgn = sign(m~ - theta) in {-1,0,1};
  s1 = (sgn+1)/2 is folded into layer 2 as W2/2 plus a constant row
  (rank-1 ones matmul), so no extra elementwise pass materializes s1.
"""

import math

import numpy as np

B, F, NN, T, H, C = 4096, 32, 8, 100, 512, 8
D = F * NN            # 256
NCORES = 8
BC = B // NCORES      # 512 batch rows per core
TAU = 20.0
BETA = float(np.exp(-1.0 / TAU))
TW = 1.0 / NN
TC = 4                # timesteps per noise DMA batch

_cache = {}


def _split_multi_waits(nc):
    """This walrus build encodes only ONE sync wait per instruction; hoist
    extra waits into preceding NoOps on the same (in-order) engine queue."""
    import concourse.mybir as mybir

    n = 0
    for fn in nc.m.functions:
        for bb in fn.blocks:
            new_insts = []
            for inst in bb.instructions:
                si = getattr(inst, "sync_info", None)
                if si is not None and si.on_wait and len(si.on_wait) > 1:
                    waits = list(si.on_wait)
                    for w in waits[:-1]:
                        nop = mybir.InstNoOp(
                            name=nc.get_next_instruction_name(),
                            engine=inst.engine,
                            bass_nofuse=True,
                            sync_info=mybir.SyncInfo(on_wait=[w], on_update=[]),
                        )
                        new_insts.append(nop)
                        n += 1
                    inst.sync_info = mybir.SyncInfo(
                        on_wait=[waits[-1]], on_update=list(si.on_update or [])
                    )
                new_insts.append(inst)
            bb.instructions[:] = new_insts
    return n


def _build():
    import concourse.bass as bass
    import concourse.mybir as mybir
    from concourse.masks import make_identity
    from concourse.tile import TileContext

    f32 = mybir.dt.float32
    A = mybir.AluOpType
    ACTF = mybir.ActivationFunctionType

    nc = bass.Bass()
    noise_d = nc.dram_tensor("noise", [BC, T, D], f32, kind="ExternalInput")
    w1t_d = nc.dram_tensor("w1t", [128, 2 * H], f32, kind="ExternalInput")
    v_d = nc.dram_tensor("v", [128, 4 * C], f32, kind="ExternalInput")
    c2_d = nc.dram_tensor("c2", [1, C], f32, kind="ExternalInput")
    pt_d = nc.dram_tensor("pt", [128, 2 * BC], f32, kind="ExternalInput")
    bias_d = nc.dram_tensor("biastbl", [128, T], f32, kind="ExternalInput")
    sgn2_d = nc.dram_tensor("sgn2out", [T, C, BC], f32, kind="ExternalOutput")

    with TileContext(nc) as tc:
        with (
            tc.tile_pool(name="static", bufs=1) as stat,
            tc.tile_pool(name="noise", bufs=2) as npool,
            tc.tile_pool(name="wts", bufs=2) as wpool,
            tc.tile_pool(name="sgns", bufs=2) as gpool,
            tc.tile_pool(name="spks", bufs=2) as spool,
            tc.tile_pool(name="psA", bufs=1, space="PSUM") as psA,
            tc.tile_pool(name="psB", bufs=1, space="PSUM") as psB,
            tc.tile_pool(name="psC", bufs=1, space="PSUM") as psC,
        ):
            # ---- static tiles ----
            w1t_base = stat.tile([128, 2 * H], f32, tag="w1tb")
            nc.sync.dma_start(w1t_base[:], w1t_d[:])
            v_base = stat.tile([128, 4 * C], f32, tag="vb")
            nc.sync.dma_start(v_base[:], v_d[:])
            c2_base = stat.tile([1, C], f32, tag="c2b")
            nc.sync.dma_start(c2_base[:], c2_d[:])
            pt_sb = stat.tile([128, 2 * BC], f32, tag="pt")
            nc.sync.dma_start(pt_sb[:], pt_d[:])
            bias_tbl = stat.tile([128, T], f32, tag="bias")
            nc.sync.dma_start(bias_tbl[:], bias_d[:])
            ident = stat.tile([128, 128], f32, tag="ident")
            make_identity(nc, ident[:])
            ones_sb = stat.tile([1, BC], f32, tag="ones")
            nc.gpsimd.memset(ones_sb[:], 1.0)

            # ---- persistent PSUM state ----
            m1_ps = psA.tile([128, 4 * BC], f32)   # 4 banks: [h0, m*BC + b]
            nt_ps = psB.tile([128, 2 * BC], f32)   # 2 banks: [d0, k*BC + b]
            m2_ps = psC.tile([C, BC], f32)         # 1 bank

            noise3d = noise_d[:].rearrange("b t d -> b (t d)")

            for tb in range(T // TC):
                nchunks = []
                for j in range(4):
                    ch = npool.tile([128, TC * D], f32, tag=f"n{j}")
                    nc.sync.dma_start(
                        ch[:],
                        noise3d[j * 128:(j + 1) * 128,
                                tb * TC * D:(tb + 1) * TC * D],
                    )
                    nchunks.append(ch)

                for tt in range(TC):
                    t = tb * TC + tt
                    scale = float(math.exp(t / 20.0))  # beta^-t

                    # per-step weight rescales (ACT copy with imm scale)
                    w1t_t = wpool.tile([128, 2 * H], f32, tag="w1t")
                    nc.scalar.activation(w1t_t[:], w1t_base[:], ACTF.Copy, scale=scale)
                    v_t = wpool.tile([128, 4 * C], f32, tag="v")
                    nc.scalar.activation(v_t[:], v_base[:], ACTF.Copy, scale=scale)
                    c2_t = wpool.tile([1, C], f32, tag="c2")
                    nc.scalar.activation(c2_t[:], c2_base[:], ACTF.Copy, scale=scale)

                    # noise transposes: (128b,128d) -> (128d,128b) into PSUM
                    for k in range(2):
                        for j in range(4):
                            nc.tensor.transpose(
                                nt_ps[:, k * BC + j * 128:k * BC + (j + 1) * 128],
                                nchunks[j][:, tt * D + k * 128:tt * D + (k + 1) * 128],
                                ident[:],
                            )

                    # spikes: (noiseT < pT) in {0,1}
                    spk = spool.tile([128, 2 * BC], f32, tag="spk")
                    nc.vector.tensor_tensor(
                        out=spk[:], in0=nt_ps[:], in1=pt_sb[:], op=A.is_lt
                    )

                    # layer 1: m1~ += spk.T-blocks @ (beta^-t W1T)
                    for k in range(2):
                        for m in range(4):
                            nc.tensor.matmul(
                                m1_ps[:, m * BC:(m + 1) * BC],
                                w1t_t[:, k * H + m * 128:k * H + (m + 1) * 128],
                                spk[:, k * BC:(k + 1) * BC],
                                start=(t == 0 and k == 0),
                                stop=(t == T - 1 and k == 1),
                            )

                    # sgn = Sign(m1~ - theta_t)
                    sgn = gpool.tile([128, 4 * BC], f32, tag="sgn")
                    nc.scalar.activation(
                        sgn[:], m1_ps[:], ACTF.Sign, bias=bias_tbl[:, t:t + 1]
                    )
                    # reset: m1~ *= (sgn <= 0)
                    nc.vector.scalar_tensor_tensor(
                        out=m1_ps[:], in0=sgn[:], scalar=0.0, in1=m1_ps[:],
                        op0=A.is_le, op1=A.mult,
                    )

                    # layer 2: m2~ += sgn-blocks @ (beta^-t W2T/2) + beta^-t c2
                    for k in range(4):
                        nc.tensor.matmul(
                            m2_ps[:, :],
                            v_t[:, k * C:(k + 1) * C],
                            sgn[:, k * BC:(k + 1) * BC],
                            start=(t == 0 and k == 0),
                            stop=False,
                        )
                    nc.tensor.matmul(
                        m2_ps[:, :], c2_t[:], ones_sb[:],
                        start=False, stop=(t == T - 1),
                    )

                    # sgn2 = Sign(m2~ - theta_t); reset; acc += (sgn2 > 0)
                    sgn2 = gpool.tile([C, BC], f32, tag="sgn2")
                    nc.scalar.activation(
                        sgn2[:], m2_ps[:], ACTF.Sign, bias=bias_tbl[:C, t:t + 1]
                    )
                    nc.vector.scalar_tensor_tensor(
                        out=m2_ps[:], in0=sgn2[:], scalar=0.0, in1=m2_ps[:],
                        op0=A.is_le, op1=A.mult,
                    )
                    nc.sync.dma_start(sgn2_d[t, :, :], sgn2[:])

    _split_multi_waits(nc)
    return nc


def _host_tables(spectral_indices, W1, W2):
    si = np.asarray(spectral_indices, dtype=np.float32)
    W1 = np.asarray(W1, dtype=np.float32)
    W2 = np.asarray(W2, dtype=np.float32)

    preferred = np.linspace(0.0, 1.0, NN, dtype=np.float32)
    dist2 = (si[..., None] - preferred) ** 2
    act = np.exp(-dist2 / np.float32(2.0 * TW * TW)).reshape(B, D).astype(np.float32)
    p = (act * np.float32(0.1)).astype(np.float32)          # (B, D) thresholds

    W1T = W1.T.astype(np.float32)                            # (D, H)
    w1t = np.hstack([W1T[0:128], W1T[128:256]])              # (128, 2H)
    Vb = (W2.T * np.float32(0.5)).astype(np.float32)         # (H, C)
    v = np.hstack([Vb[k * 128:(k + 1) * 128] for k in range(4)])  # (128, 4C)
    c2 = (0.5 * W2.sum(axis=1, dtype=np.float32)).astype(np.float32)[None, :]  # (1, C)

    theta = np.exp(np.arange(T, dtype=np.float64) / 20.0).astype(np.float32)
    bias_tbl = np.repeat(-theta[None, :], 128, axis=0).astype(np.float32)  # (128, T)
    return p, w1t, v, c2, bias_tbl


def kernel(spectral_indices, W1, W2, enc_noise, _want_results=False):
    from concourse.bass_utils import run_bass_kernel_spmd

    if "nc" not in _cache:
        _cache["nc"] = _build()
    nc = _cache["nc"]
    trace = globals().get("_profile_next", False)
    globals()["_profile_next"] = False

    p, w1t, v, c2, bias_tbl = _host_tables(spectral_indices, W1, W2)
    noise = np.ascontiguousarray(np.asarray(enc_noise, dtype=np.float32))

    in_maps = []
    for core in range(NCORES):
        sl = slice(core * BC, (core + 1) * BC)
        pT = p[sl].T                                          # (D, BC)
        pt = np.ascontiguousarray(np.hstack([pT[0:128], pT[128:256]]))  # (128, 2BC)
        in_maps.append({
            "noise": np.ascontiguousarray(noise[sl]),
            "w1t": w1t, "v": v, "c2": c2, "pt": pt, "biastbl": bias_tbl,
        })

    res = run_bass_kernel_spmd(nc, in_maps, core_ids=list(range(NCORES)), trace=trace)
    if _want_results:
        return res

    out = np.empty((B, C), dtype=np.float32)
    for core in range(NCORES):
        sgn2 = res.results[core]["sgn2out"]                   # (T, C, BC)
        acc = (sgn2 > 0).sum(axis=0, dtype=np.float32)        # (C, BC)
        out[core * BC:(core + 1) * BC, :] = acc.T / np.float32(T)
    return out


# revision 19
# speedup vs baseline: 88.5469x; 88.5469x over previous
"""Trainium2 Bass kernel for the CropHealthClassifier SNN.

Population-encoded Bernoulli spike trains -> 2-layer LIF SNN over T=100
timesteps -> output firing rates (B, C). Pure data parallel over 8 cores.

Key design points (v2):
- Membrane state lives in PSUM; the PE accumulates layer inputs directly onto
  it (matmul start=False; ACT/DVE writes to PSUM leave has_written set, so
  accumulate-onto-modified-state works).
- Weights are split host-side into bf16-representable hi + lo parts but run
  as float32r matmuls: f32r streams at full PE rate (4x faster than fp32's
  2-half-speed-pass mode) and passes bf16-representable mantissas exactly,
  so hi+lo accumulation reproduces full fp32 weight precision (~2^-17),
  which the spike dynamics verify bit-exact against the reference.
- Spikes {0,1} and sign outputs {-1,0,1} are exact in any dtype, so they
  ride f32r matmuls losslessly.
- The per-step beta decay + threshold reset is one Sign activation (ACT),
  one {0,beta} gate (GPSIMD tensor_scalar), and one in-place PSUM multiply
  (DVE) per membrane bank; layer 2 consumes the sign tensor directly via
  s1 = (sgn+1)/2 folded into W2/2 plus a rank-1 ones row.
- Spikes enter layer 1 transposed (D, B); the (B, D) -> (D, B) transpose is
  done on the noise via PE transpose-mode matmuls, and the spike compare
  (DVE) doubles as the PSUM->SBUF evacuation.
"""

import numpy as np

B, F, NN, T, H, C = 4096, 32, 8, 100, 512, 8
D = F * NN            # 256
NCORES = 8
BC = B // NCORES      # 512 batch rows per core
TAU = 20.0
BETA = float(np.exp(-1.0 / TAU))
TW = 1.0 / NN
TC = 4                # timesteps per noise DMA batch

_cache = {}


def _split_multi_waits(nc):
    """This walrus build encodes only ONE sync wait per instruction; hoist
    extra waits into preceding NoOps on the same (in-order) engine queue."""
    import concourse.mybir as mybir

    n = 0
    for fn in nc.m.functions:
        for bb in fn.blocks:
            new_insts = []
            for inst in bb.instructions:
                si = getattr(inst, "sync_info", None)
                if si is not None and si.on_wait and len(si.on_wait) > 1:
                    waits = list(si.on_wait)
                    for w in waits[:-1]:
                        nop = mybir.InstNoOp(
                            name=nc.get_next_instruction_name(),
                            engine=inst.engine,
                            bass_nofuse=True,
                            sync_info=mybir.SyncInfo(on_wait=[w], on_update=[]),
                        )
                        new_insts.append(nop)
                        n += 1
                    inst.sync_info = mybir.SyncInfo(
                        on_wait=[waits[-1]], on_update=list(si.on_update or [])
                    )
                new_insts.append(inst)
            bb.instructions[:] = new_insts
    return n


def _build(T_steps=T, repeat=1):
    import concourse.bass as bass
    import concourse.mybir as mybir
    from concourse.tile import TileContext

    f32 = mybir.dt.float32
    f32r = mybir.dt.float32r
    A = mybir.AluOpType
    ACTF = mybir.ActivationFunctionType

    nc = bass.Bass()
    noise_d = nc.dram_tensor("noise", [BC, T, D], f32, kind="ExternalInput")
    w1t_d = nc.dram_tensor("w1t", [128, 4 * H], f32r, kind="ExternalInput")
    v_d = nc.dram_tensor("v", [128, 8 * C], f32r, kind="ExternalInput")
    c2_d = nc.dram_tensor("c2", [1, 2 * C], f32r, kind="ExternalInput")
    pt_d = nc.dram_tensor("pt", [128, 2 * BC], f32, kind="ExternalInput")
    id_d = nc.dram_tensor("ident", [128, 128], f32, kind="ExternalInput")
    ones_d = nc.dram_tensor("ones", [1, BC], f32r, kind="ExternalInput")
    sgn2_d = nc.dram_tensor("sgn2out", [T, C, BC], f32, kind="ExternalOutput")

    with TileContext(nc) as tc:
        with (
            tc.tile_pool(name="static", bufs=1) as stat,
            tc.tile_pool(name="noise", bufs=2) as npool,
            tc.tile_pool(name="sgns", bufs=2) as gpool,
            tc.tile_pool(name="spks", bufs=2) as spool,
            tc.tile_pool(name="psA", bufs=1, space="PSUM") as psA,
            tc.tile_pool(name="psB", bufs=1, space="PSUM") as psB,
            tc.tile_pool(name="psC", bufs=1, space="PSUM") as psC,
        ):
            # ---- static tiles ----
            w1t_sb = stat.tile([128, 4 * H], f32r, tag="w1tb")
            nc.gpsimd.dma_start(w1t_sb[:], w1t_d[:])
            v_sb = stat.tile([128, 8 * C], f32r, tag="vb")
            nc.gpsimd.dma_start(v_sb[:], v_d[:])
            c2_sb = stat.tile([1, 2 * C], f32r, tag="c2b")
            nc.gpsimd.dma_start(c2_sb[:], c2_d[:])
            pt_sb = stat.tile([128, 2 * BC], f32, tag="pt")
            nc.gpsimd.dma_start(pt_sb[:], pt_d[:])
            ident = stat.tile([128, 128], f32, tag="ident")
            nc.gpsimd.dma_start(ident[:], id_d[:])
            ones_sb = stat.tile([1, BC], f32r, tag="ones")
            nc.gpsimd.dma_start(ones_sb[:], ones_d[:])
            neg1 = stat.tile([128, 1], f32, tag="neg1")
            nc.gpsimd.memset(neg1[:], -1.0)

            # ---- persistent PSUM state ----
            m1_ps = psA.tile([128, 4 * BC], f32)    # 4 banks: [h0, m*BC + b]
            nt_ps = psB.tile([128, 2 * BC], f32)    # 2 banks: [d0, k*BC + b]
            m2_ps = psC.tile([C, BC], f32)          # 1 bank

            noise3d = noise_d[:].rearrange("b t d -> b (t d)")

            for _rep in range(repeat):
             for tb in range(T_steps // TC):
                nchunks = []
                for j in range(4):
                    ch = npool.tile([128, TC * D], f32, tag=f"n{j}")
                    nc.sync.dma_start(
                        ch[:],
                        noise3d[j * 128:(j + 1) * 128,
                                tb * TC * D:(tb + 1) * TC * D],
                    )
                    nchunks.append(ch)

                for tt in range(TC):
                    t = tb * TC + tt

                    # noise transposes: (128b,128d) -> (128d,128b) into PSUM
                    for k in range(2):
                        for j in range(4):
                            nc.tensor.transpose(
                                nt_ps[:, k * BC + j * 128:k * BC + (j + 1) * 128],
                                nchunks[j][:, tt * D + k * 128:tt * D + (k + 1) * 128],
                                ident[:],
                            )

                    # spikes: (noiseT < pT) in {0,1}, per d-tile
                    spk = spool.tile([128, 2 * BC], f32r, tag="spk")
                    for k in range(2):
                        nc.vector.tensor_tensor(
                            out=spk[:, k * BC:(k + 1) * BC],
                            in0=nt_ps[:, k * BC:(k + 1) * BC],
                            in1=pt_sb[:, k * BC:(k + 1) * BC], op=A.is_lt
                        )

                    # layer 1: m1 += spk.T-blocks @ (W1T_hi + W1T_lo), f32r
                    for m in range(4):
                        for k in range(2):
                            for p in range(2):
                                nc.tensor.matmul(
                                    m1_ps[:, m * BC:(m + 1) * BC],
                                    w1t_sb[:, (2 * p + k) * H + m * 128:
                                           (2 * p + k) * H + (m + 1) * 128],
                                    spk[:, k * BC:(k + 1) * BC],
                                    start=(t == 0 and k == 0 and p == 0
                                           and _rep == 0),
                                    stop=False,
                                )

                    # sgn = Sign(m1 - 1) per bank (ACT); gate g = (sgn<=0)*beta
                    # (GPSIMD); reset+decay m1 *= g (DVE)
                    sgn = gpool.tile([128, 4 * BC], f32r, tag="sgn")
                    g = gpool.tile([128, 4 * BC], f32, tag="g")
                    for m in range(4):
                        sl = slice(m * BC, (m + 1) * BC)
                        nc.scalar.activation(
                            sgn[:, sl], m1_ps[:, sl], ACTF.Sign, bias=neg1[:]
                        )
                    for m in range(4):
                        sl = slice(m * BC, (m + 1) * BC)
                        nc.vector.tensor_scalar(
                            out=g[:, sl], in0=sgn[:, sl], scalar1=0.0,
                            scalar2=BETA, op0=A.is_le, op1=A.mult,
                        )
                    for m in range(4):
                        sl = slice(m * BC, (m + 1) * BC)
                        nc.vector.tensor_tensor(
                            out=m1_ps[:, sl], in0=m1_ps[:, sl], in1=g[:, sl],
                            op=A.mult,
                        )

                    # layer 2: m2 += sgn @ (V_hi + V_lo) + (c2_hi + c2_lo)
                    for p in range(2):
                        for k in range(4):
                            nc.tensor.matmul(
                                m2_ps[:, :],
                                v_sb[:, (4 * p + k) * C:(4 * p + k + 1) * C],
                                sgn[:, k * BC:(k + 1) * BC],
                                start=(t == 0 and k == 0 and p == 0
                                       and _rep == 0),
                                stop=False,
                            )
                    for p in range(2):
                        nc.tensor.matmul(
                            m2_ps[:, :], c2_sb[:, p * C:(p + 1) * C],
                            ones_sb[:],
                            start=False,
                            stop=(t == T_steps - 1 and p == 1),
                        )

                    # sgn2 = Sign(m2 - 1); gate2; m2 *= g2; ship sgn2 out
                    sgn2 = gpool.tile([C, BC], f32, tag="sgn2")
                    nc.scalar.activation(
                        sgn2[:], m2_ps[:], ACTF.Sign, bias=neg1[:C]
                    )
                    g2 = gpool.tile([C, BC], f32, tag="g2")
                    nc.vector.tensor_scalar(
                        out=g2[:], in0=sgn2[:], scalar1=0.0, scalar2=BETA,
                        op0=A.is_le, op1=A.mult,
                    )
                    nc.vector.tensor_tensor(
                        out=m2_ps[:], in0=m2_ps[:], in1=g2[:], op=A.mult,
                    )
                    nc.sync.dma_start(sgn2_d[t, :, :], sgn2[:])

    _split_multi_waits(nc)
    return nc


def _bf16_split(x):
    import ml_dtypes
    hi = x.astype(ml_dtypes.bfloat16).astype(np.float32)
    lo = (x - hi).astype(ml_dtypes.bfloat16).astype(np.float32)
    return hi, lo


def _host_tables(spectral_indices, W1, W2):
    si = np.asarray(spectral_indices, dtype=np.float32)
    W1 = np.asarray(W1, dtype=np.float32)
    W2 = np.asarray(W2, dtype=np.float32)

    preferred = np.linspace(0.0, 1.0, NN, dtype=np.float32)
    dist2 = (si[..., None] - preferred) ** 2
    act = np.exp(-dist2 / np.float32(2.0 * TW * TW)).reshape(B, D).astype(np.float32)
    p = (act * np.float32(0.1)).astype(np.float32)          # (B, D) thresholds

    W1T = W1.T.astype(np.float32)                            # (D, H)
    w1h, w1l = _bf16_split(W1T)
    w1t = np.hstack([w1h[0:128], w1h[128:256], w1l[0:128], w1l[128:256]])

    Vb = (W2.T * np.float32(0.5)).astype(np.float32)         # (H, C)
    vh, vl = _bf16_split(Vb)
    v = np.hstack([vh[k * 128:(k + 1) * 128] for k in range(4)]
                  + [vl[k * 128:(k + 1) * 128] for k in range(4)])

    c2 = (0.5 * W2.sum(axis=1, dtype=np.float32)).astype(np.float32)[None, :]
    c2h, c2l = _bf16_split(c2)
    c2hl = np.hstack([c2h, c2l])                             # (1, 2C)
    return p, w1t, v, c2hl


def kernel(spectral_indices, W1, W2, enc_noise, _want_results=False):
    from concourse.bass_utils import run_bass_kernel_spmd

    if "nc" not in _cache:
        _cache["nc"] = _build()
    nc = _cache["nc"]
    trace = globals().get("_profile_next", False)
    globals()["_profile_next"] = False

    p, w1t, v, c2hl = _host_tables(spectral_indices, W1, W2)
    noise = np.ascontiguousarray(np.asarray(enc_noise, dtype=np.float32))

    in_maps = []
    for core in range(NCORES):
        sl = slice(core * BC, (core + 1) * BC)
        pT = p[sl].T                                          # (D, BC)
        pt = np.ascontiguousarray(np.hstack([pT[0:128], pT[128:256]]))
        in_maps.append({
            "noise": np.ascontiguousarray(noise[sl]),
            "w1t": w1t, "v": v, "c2": c2hl, "pt": pt,
            "ident": np.eye(128, dtype=np.float32),
            "ones": np.ones((1, BC), dtype=np.float32),
        })

    res = run_bass_kernel_spmd(nc, in_maps, core_ids=list(range(NCORES)), trace=trace)
    if _want_results:
        return res

    out = np.empty((B, C), dtype=np.float32)
    for core in range(NCORES):
        sgn2 = res.results[core]["sgn2out"]                   # (T, C, BC)
        acc = (sgn2 > 0).sum(axis=0, dtype=np.float32)        # (C, BC)
        out[core * BC:(core + 1) * BC, :] = acc.T / np.float32(T)
    return out
